# revision 4
# baseline (speedup 1.0000x reference)
"""Trainium2 Bass kernel for nn_AttentionalSpikingSSMLayer.

Model (reference semantics): a T-step scan; per step t:
    state_transition = h @ A.T
    q = h @ Wq.T + bq                  (queries from the spiking state)
    kv = x_t @ Wkv.T + bkv             (keys/values from the input)
    att = softmax(q k^T / sqrt(dh)) v  (attention over N = B*S, per head)
    state_update = state_transition + att @ Wo.T + bo
    h, v_mem_s, thr_s = LIF(state_update)          # binary spikes
    out_t, v_mem_o, thr_o = LIF(h @ C.T)           # binary spikes

Key structural algebra (exact, holds for ANY input values of these shapes):
  h0 = 0, and x enters only through k/v which are *reduced over* by
  attention.  Therefore every row n = (b,s) of the state performs the
  identical computation: h(t), v_mem(t) and the outputs are constant
  across (b, s) for all t.  The recurrence collapses to a single
  64/512-dim trajectory.

  Further, while no state spike fires, h(t) == 0, so q(t) == bq for all
  t.  The kernel exploits this speculatively: it computes, entirely on
  device, the attention sums for q = bq for all T steps in parallel
  (sharded over the N dimension across 8 cores, one small AllReduce),
  then the collapsed membrane-potential trajectory, and emits per-step
  spike MARGINS (v_pot - thr).  If every margin is safely negative the
  hypothesis h == 0 is *proved* (by induction over t), the output is
  exactly zero, and the device-written zero tensor is the exact answer.
  If any margin is within eps of firing (or non-finite), the host falls
  back to a faithful sequential recompute of the collapsed recurrence.
"""

import math
import numpy as np

import concourse.bass as bass
import concourse.tile as tile
from concourse import bacc, mybir
from concourse import bass_utils

F32 = mybir.dt.float32

B, T, S, D = 8, 16, 256, 512
DS, H = 64, 4
DH = DS // H
N = B * S
NCORES = 8
MSH = N // NCORES          # 256 keys per core
TAU = 2.0
MEM_DECAY = math.exp(-1.0 / TAU)
ADAPT_STRENGTH = 0.1
TARGET_RATE = 0.02
THR_MIN = 0.5
EPS_MARGIN = 1e-3          # conservative spike-detection margin

_CACHE = {}


def _build_module():
    """Build + compile the 8-core Bass module once per process."""
    if "nc" in _CACHE:
        return _CACHE["nc"]

    nc = bacc.Bacc("TRN2", target_bir_lowering=False, debug=False,
                   num_devices=NCORES)

    xt = nc.dram_tensor("xt", [T, D, MSH], F32, kind="ExternalInput").ap()
    wkvt = nc.dram_tensor("wkvt", [D, 2 * DS], F32, kind="ExternalInput").ap()
    bkv = nc.dram_tensor("bkv", [2 * DS, 1], F32, kind="ExternalInput").ap()
    qblk = nc.dram_tensor("qblk", [DS, H], F32, kind="ExternalInput").ap()
    patt = nc.dram_tensor("patt", [H, DS], F32, kind="ExternalInput").ap()
    wotbo = nc.dram_tensor("wotbo", [DS + 1, DS], F32, kind="ExternalInput").ap()
    lmat = nc.dram_tensor("lmat", [T, T], F32, kind="ExternalInput").ap()
    tvals = nc.dram_tensor("tvals", [T, 1], F32, kind="ExternalInput").ap()
    ones16 = nc.dram_tensor("ones16", [1, T], F32, kind="ExternalInput").ap()
    ts0row = nc.dram_tensor("ts0row", [1, DS], F32, kind="ExternalInput").ap()

    out = nc.dram_tensor("out", [T, S, D], F32, kind="ExternalOutput").ap()
    margin = nc.dram_tensor("margin", [T, DS], F32, kind="ExternalOutput").ap()

    with tile.TileContext(nc) as tc:
        with tc.tile_pool(name="const", bufs=1) as cpool, \
             tc.tile_pool(name="work", bufs=3) as wpool, \
             tc.tile_pool(name="ps", bufs=2, space="PSUM") as ps, \
             tc.tile_pool(name="ps2", bufs=1, space="PSUM") as pss, \
             tc.tile_pool(name="dram", bufs=2, space="DRAM") as dpool:

            # ---- constants ----
            t_wkvt = cpool.tile([128, 4, 2 * DS], F32)
            nc.sync.dma_start(t_wkvt[:], wkvt.rearrange("(a p) m -> p a m", p=128))
            t_bkv = cpool.tile([2 * DS, 1], F32)
            nc.sync.dma_start(t_bkv[:], bkv[:])
            t_qblk = cpool.tile([DS, H], F32)
            nc.sync.dma_start(t_qblk[:], qblk[:])
            t_patt = cpool.tile([H, DS], F32)
            nc.sync.dma_start(t_patt[:], patt[:])
            t_wotbo = cpool.tile([DS + 1, DS], F32)
            nc.sync.dma_start(t_wotbo[:], wotbo[:])
            t_lmat = cpool.tile([T, T], F32)
            nc.sync.dma_start(t_lmat[:], lmat[:])
            t_tvals = cpool.tile([T, 1], F32)
            nc.sync.dma_start(t_tvals[:], tvals[:])
            t_ones16 = cpool.tile([1, T], F32)
            nc.sync.dma_start(t_ones16[:], ones16[:])
            t_ts0 = cpool.tile([1, DS], F32)
            nc.sync.dma_start(t_ts0[:], ts0row[:])

            # ---- output zeros (exact spikes when the h==0 margin holds) ----
            zt = cpool.tile([128, 2, D], F32)
            nc.vector.memset(zt[:], 0.0)
            for t in range(T):
                nc.sync.dma_start(
                    out[t].rearrange("(a p) d -> p a d", p=128), zt[:])

            # ---- per-step attention partials over this core's key shard ----
            attsums = cpool.tile([DS, T], F32)   # sum_m w * v   (per feature)
            wsums = cpool.tile([H, T], F32)      # sum_m w       (per head)
            for t in range(T):
                xt_t = wpool.tile([128, 4, MSH], F32, tag="xt")
                nc.sync.dma_start(xt_t[:], xt[t].rearrange("(a p) m -> p a m", p=128))
                kv_ps = ps.tile([128, MSH], F32, tag="kv")
                for a in range(4):
                    nc.tensor.matmul(kv_ps[:], t_wkvt[:, a, :], xt_t[:, a, :],
                                     start=(a == 0), stop=(a == 3))
                kvT = wpool.tile([128, MSH], F32, tag="kvT")
                nc.vector.tensor_scalar(kvT[:], kv_ps[:], t_bkv[:], None,
                                        op0=mybir.AluOpType.add)
                # scores (head, m) for q = bq (valid while h == 0), scale folded
                sc_ps = ps.tile([H, MSH], F32, tag="sc")
                nc.tensor.matmul(sc_ps[:], t_qblk[:], kvT[0:DS, :],
                                 start=True, stop=True)
                w = wpool.tile([H, MSH], F32, tag="w")
                nc.scalar.activation(w[:], sc_ps[:],
                                     mybir.ActivationFunctionType.Exp)
                # replicate head weights across the 16 features of each head
                wrep_ps = ps.tile([DS, MSH], F32, tag="wrep")
                nc.tensor.matmul(wrep_ps[:], t_patt[:], w[:], start=True, stop=True)
                scr = wpool.tile([DS, MSH], F32, tag="scr")
                nc.vector.tensor_mul(scr[:], kvT[DS:2 * DS, :], wrep_ps[:])
                nc.vector.reduce_sum(out=attsums[:, t:t + 1], in_=scr[:],
                                     axis=mybir.AxisListType.X)
                nc.vector.reduce_sum(out=wsums[:, t:t + 1], in_=w[:],
                                     axis=mybir.AxisListType.X)

            # ---- one AllReduce of all per-step partials ----
            arin = dpool.tile([DS + H, T], F32)
            arout = dpool.tile([DS + H, T], F32, addr_space="Shared")
            nc.sync.dma_start(arin[0:DS, :], attsums[:])
            nc.sync.dma_start(arin[DS:DS + H, :], wsums[:])
            nc.gpsimd.collective_compute(
                "AllReduce", mybir.AluOpType.add,
                replica_groups=[list(range(NCORES))],
                ins=[arin.opt()], outs=[arout.opt()])
            arres = wpool.tile([DS + H, T], F32, tag="arres")
            nc.sync.dma_start(arres[:], arout[:])

            # ---- collapsed trajectory + spike margins (replicated) ----
            recip = wpool.tile([H, T], F32, tag="recip")
            nc.vector.reciprocal(recip[:], arres[DS:DS + H, :])
            wrec_ps = pss.tile([DS, T], F32, tag="p2")
            nc.tensor.matmul(wrec_ps[:], t_patt[:], recip[:], start=True, stop=True)
            att_n = cpool.tile([DS + 1, T], F32)
            nc.vector.memset(att_n[DS:DS + 1, :], 1.0)
            nc.vector.tensor_mul(att_n[0:DS, :], arres[0:DS, :], wrec_ps[:])
            # su^T = att_n^T @ [Wo^T; bo]  -> (T, DS)
            su_ps = pss.tile([T, DS], F32, tag="p2")
            nc.tensor.matmul(su_ps[:], att_n[:], t_wotbo[:], start=True, stop=True)
            suT = wpool.tile([T, DS], F32, tag="suT")
            nc.vector.tensor_copy(suT[:], su_ps[:])
            # v_pot(t) = sum_tau decay^(t-tau) su(tau): upper-tri decay matmul
            vp_ps = pss.tile([T, DS], F32, tag="vp")
            nc.tensor.matmul(vp_ps[:], t_lmat[:], suT[:], start=True, stop=True)
            # threshold trajectory under the no-spike hypothesis
            thr_ps = pss.tile([T, DS], F32, tag="p2")
            nc.tensor.matmul(thr_ps[:], t_ones16[:], t_ts0[:], start=True, stop=True)
            thr_sb = wpool.tile([T, DS], F32, tag="thr_sb")
            nc.vector.tensor_scalar(thr_sb[:], thr_ps[:], t_tvals[:], THR_MIN,
                                    op0=mybir.AluOpType.subtract,
                                    op1=mybir.AluOpType.max)
            nc.vector.tensor_copy(thr_sb[0:1, :], t_ts0[:])  # t=0 uses thr_s0 raw
            marg = wpool.tile([T, DS], F32, tag="marg")
            nc.vector.tensor_sub(marg[:], vp_ps[:], thr_sb[:])
            nc.sync.dma_start(margin[:], marg[:])

    nc.compile()
    _CACHE["nc"] = nc
    return nc


def _softmax_f32(s):
    m = s.max()
    e = np.exp(s - m, dtype=np.float32)
    return e / e.sum(dtype=np.float32)


def _fallback(x, A, C, Wq, bq, Wkv, bkv, Wo, bo, thr_s0, thr_o0):
    """Faithful host recompute of the collapsed recurrence (rows of the
    state are identical across n = (b, s) for any input, by induction
    from h0 = 0)."""
    x = np.asarray(x, np.float32)
    xt_all = np.moveaxis(x, 1, 0).reshape(T, N, D)
    decay = np.float32(MEM_DECAY)
    h = np.zeros(DS, np.float32)
    sv = np.zeros(DS, np.float32)
    ov = np.zeros(D, np.float32)
    ts = np.asarray(thr_s0, np.float32).copy()
    to = np.asarray(thr_o0, np.float32).copy()
    outs = np.zeros((T, D), np.float32)
    scale = np.float32(1.0 / math.sqrt(DH))
    for t in range(T):
        kv = xt_all[t] @ np.asarray(Wkv, np.float32).T + np.asarray(bkv, np.float32)
        k = kv[:, :DS].reshape(N, H, DH)
        v = kv[:, DS:].reshape(N, H, DH)
        q = (h @ np.asarray(Wq, np.float32).T + np.asarray(bq, np.float32)).reshape(H, DH)
        att = np.zeros((H, DH), np.float32)
        for hh in range(H):
            s = (k[:, hh, :] @ q[hh]) * scale
            w = _softmax_f32(s)
            att[hh] = w @ v[:, hh, :]
        su = h @ np.asarray(A, np.float32).T + att.reshape(DS) @ np.asarray(Wo, np.float32).T + np.asarray(bo, np.float32)
        vp = sv * decay + su
        spk = (vp - ts >= 0).astype(np.float32)
        sv = vp * (1 - spk)
        ts = np.maximum(ts + np.float32(ADAPT_STRENGTH) * (spk.mean(dtype=np.float32) - np.float32(TARGET_RATE)), np.float32(THR_MIN))
        h = spk
        op = h @ np.asarray(C, np.float32).T
        vpo = ov * decay + op
        spko = (vpo - to >= 0).astype(np.float32)
        ov = vpo * (1 - spko)
        to = np.maximum(to + np.float32(ADAPT_STRENGTH) * (spko.mean(dtype=np.float32) - np.float32(TARGET_RATE)), np.float32(THR_MIN))
        outs[t] = spko
    # broadcast the (identical) rows to the full output
    full = np.broadcast_to(outs[None, :, None, :], (B, T, S, D))
    return np.ascontiguousarray(full, dtype=np.float32)


def kernel(x, A, C, Wq, bq, Wkv, bkv, Wo, bo, thr_s0, thr_o0):
    x = np.ascontiguousarray(np.asarray(x, np.float32))
    bq = np.asarray(bq, np.float32)
    Wkv_ = np.asarray(Wkv, np.float32)
    bkv_ = np.asarray(bkv, np.float32)
    Wo_ = np.asarray(Wo, np.float32)
    bo_ = np.asarray(bo, np.float32)
    thr_s0 = np.asarray(thr_s0, np.float32)
    thr_o0 = np.asarray(thr_o0, np.float32)

    nc = _build_module()

    # host-side input marshaling (layout only)
    scale = np.float32(1.0 / math.sqrt(DH))
    qblk = np.zeros((DS, H), np.float32)
    for j in range(DS):
        qblk[j, j // DH] = bq[j] * scale
    patt = np.zeros((H, DS), np.float32)
    for j in range(DS):
        patt[j // DH, j] = 1.0
    wotbo = np.concatenate([Wo_.T, bo_[None, :]], axis=0).astype(np.float32)
    lmat = np.zeros((T, T), np.float32)
    for tau in range(T):
        for t in range(tau, T):
            lmat[tau, t] = MEM_DECAY ** (t - tau)
    tvals = (np.float32(-ADAPT_STRENGTH) * np.float32(-TARGET_RATE)
             * np.arange(T, dtype=np.float32)).reshape(T, 1)
    consts = {
        "wkvt": np.ascontiguousarray(Wkv_.T),
        "bkv": bkv_.reshape(2 * DS, 1),
        "qblk": qblk,
        "patt": patt,
        "wotbo": wotbo,
        "lmat": lmat,
        "tvals": tvals,
        "ones16": np.ones((1, T), np.float32),
        "ts0row": thr_s0.reshape(1, DS),
    }
    in_maps = []
    for c in range(NCORES):
        m = dict(consts)
        m["xt"] = np.ascontiguousarray(x[c].transpose(0, 2, 1))
        in_maps.append(m)

    res = bass_utils.run_bass_kernel_spmd(nc, in_maps, core_ids=list(range(NCORES)))

    margins = np.stack([r["margin"] for r in res.results])
    need_fallback = (
        not np.isfinite(margins).all()
        or float(np.nanmax(margins)) >= -EPS_MARGIN
        or float(thr_o0.min()) <= EPS_MARGIN
    )
    if need_fallback:
        return _fallback(x, A, C, Wq, bq, Wkv, bkv, Wo, bo, thr_s0, thr_o0)

    # spike-free trajectory proved: output is the device-written zeros
    out = np.stack([r["out"] for r in res.results])  # (B, T, S, D)
    return np.ascontiguousarray(out, dtype=np.float32)


# revision 6
# speedup vs baseline: 1.9951x; 1.9951x over previous
"""Trainium2 Bass kernel for nn_AttentionalSpikingSSMLayer.

Model (reference semantics): a T-step scan; per step t:
    state_transition = h @ A.T
    q = h @ Wq.T + bq                  (queries from the spiking state)
    kv = x_t @ Wkv.T + bkv             (keys/values from the input)
    att = softmax(q k^T / sqrt(dh)) v  (attention over N = B*S, per head)
    state_update = state_transition + att @ Wo.T + bo
    h, v_mem_s, thr_s = LIF(state_update)          # binary spikes
    out_t, v_mem_o, thr_o = LIF(h @ C.T)           # binary spikes

Key structural algebra (exact, holds for ANY input values of these shapes):
  h0 = 0, and x enters only through k/v which are *reduced over* by
  attention.  Therefore every row n = (b,s) of the state performs the
  identical computation: h(t), v_mem(t) and the outputs are constant
  across (b, s) for all t.  The recurrence collapses to a single
  64/512-dim trajectory.

  Further, while no state spike fires, h(t) == 0, so q(t) == bq for all
  t.  The kernel exploits this speculatively: on device it computes the
  per-step attention sums for q = bq for all T steps in parallel
  (sharded over the N dimension across the 8 cores) and writes each
  core's partial sums; the host adds the 8 partials and verifies the
  no-spike hypothesis via the membrane-potential margins (v_pot - thr).
  If every margin is safely negative the hypothesis h == 0 is *proved*
  (by induction over t), the output is exactly zero, and the
  device-written zero tensor is the exact answer.  If any margin is
  within eps of firing (or non-finite), the host falls back to a
  faithful sequential recompute of the collapsed recurrence.

  The kv / scores / head-broadcast matmuls run in float32r (single-pass
  fp32, ~1e-4 relative rounding): the verification margin is O(1), so
  this cannot change any spike decision that the eps-guard would not
  already route to the exact fallback.
"""

import math
import numpy as np

import concourse.bass as bass
import concourse.tile as tile
from concourse import bacc, mybir
from concourse import bass_utils

F32 = mybir.dt.float32
F32R = mybir.dt.float32r

B, T, S, D = 8, 16, 256, 512
DS, H = 64, 4
DH = DS // H
N = B * S
NCORES = 8
MSH = N // NCORES          # 256 keys per core
TAU = 2.0
MEM_DECAY = math.exp(-1.0 / TAU)
ADAPT_STRENGTH = 0.1
TARGET_RATE = 0.02
THR_MIN = 0.5
EPS_MARGIN = 1e-3          # conservative spike-detection margin

_CACHE = {}


def _build_module():
    """Build + compile the 8-core Bass module once per process."""
    if "nc" in _CACHE:
        return _CACHE["nc"]

    nc = bacc.Bacc("TRN2", target_bir_lowering=False, debug=False,
                   num_devices=NCORES)

    xt = nc.dram_tensor("xt", [T, D, MSH], F32, kind="ExternalInput").ap()
    wkvt = nc.dram_tensor("wkvt", [D, 2 * DS], F32, kind="ExternalInput").ap()
    bkv = nc.dram_tensor("bkv", [2 * DS, 1], F32, kind="ExternalInput").ap()
    qblk = nc.dram_tensor("qblk", [DS, H], F32, kind="ExternalInput").ap()
    patt = nc.dram_tensor("patt", [H, DS], F32, kind="ExternalInput").ap()

    out = nc.dram_tensor("out", [T, S, D], F32, kind="ExternalOutput").ap()
    partials = nc.dram_tensor("partials", [DS + H, T], F32,
                              kind="ExternalOutput").ap()

    with tile.TileContext(nc) as tc:
        with tc.tile_pool(name="const", bufs=1) as cpool, \
             tc.tile_pool(name="work", bufs=3) as wpool, \
             tc.tile_pool(name="ps", bufs=2, space="PSUM") as ps:

            # ---- constants (rounded to f32r where they feed matmuls) ----
            t_wkvt = cpool.tile([128, 4, 2 * DS], F32R)
            nc.gpsimd.dma_start(t_wkvt[:], wkvt.rearrange("(a p) m -> p a m", p=128))
            t_bkv = cpool.tile([2 * DS, 1], F32)
            nc.sync.dma_start(t_bkv[:], bkv[:])
            t_qblk = cpool.tile([DS, H], F32R)
            nc.gpsimd.dma_start(t_qblk[:], qblk[:])
            t_patt = cpool.tile([H, DS], F32R)
            nc.gpsimd.dma_start(t_patt[:], patt[:])

            zt = cpool.tile([128, 2, D], F32)
            nc.vector.memset(zt[:], 0.0)

            attsums = cpool.tile([DS, T], F32)   # sum_m w * v   (per feature)
            wsums = cpool.tile([H, T], F32)      # sum_m w       (per head)
            for t in range(T):
                # zero this step's output slab on the gpsimd DMA ring so it
                # overlaps the sync-ring input loads
                nc.sync.dma_start(
                    out[t].rearrange("(a p) d -> p a d", p=128), zt[:])

                xt_t = wpool.tile([128, 4, MSH], F32R, tag="xt")
                nc.gpsimd.dma_start(xt_t[:], xt[t].rearrange("(a p) m -> p a m", p=128))
                kv_ps = ps.tile([128, MSH], F32, tag="kv")
                for a in range(4):
                    nc.tensor.matmul(kv_ps[:], t_wkvt[:, a, :], xt_t[:, a, :],
                                     start=(a == 0), stop=(a == 3))
                kvT = wpool.tile([128, MSH], F32R, tag="kvT")
                nc.vector.tensor_scalar(kvT[:], kv_ps[:], t_bkv[:], None,
                                        op0=mybir.AluOpType.add)
                # scores (head, m) for q = bq (valid while h == 0), scale folded
                sc_ps = ps.tile([H, MSH], F32, tag="sc")
                nc.tensor.matmul(sc_ps[:], t_qblk[:], kvT[0:DS, :],
                                 start=True, stop=True)
                w = wpool.tile([H, MSH], F32R, tag="w")
                nc.scalar.activation(w[:], sc_ps[:],
                                     mybir.ActivationFunctionType.Exp)
                # replicate head weights across the 16 features of each head
                wrep_ps = ps.tile([DS, MSH], F32, tag="wrep")
                nc.tensor.matmul(wrep_ps[:], t_patt[:], w[:], start=True, stop=True)
                scr = wpool.tile([DS, MSH], F32, tag="scr")
                nc.vector.tensor_mul(scr[:], kvT[DS:2 * DS, :], wrep_ps[:])
                nc.vector.reduce_sum(out=attsums[:, t:t + 1], in_=scr[:],
                                     axis=mybir.AxisListType.X)
                nc.vector.reduce_sum(out=wsums[:, t:t + 1], in_=w[:],
                                     axis=mybir.AxisListType.X)

            nc.sync.dma_start(partials[0:DS, :], attsums[:])
            nc.sync.dma_start(partials[DS:DS + H, :], wsums[:])

    nc.compile()
    _CACHE["nc"] = nc
    return nc


def _softmax_f32(s):
    m = s.max()
    e = np.exp(s - m, dtype=np.float32)
    return e / e.sum(dtype=np.float32)


def _fallback(x, A, C, Wq, bq, Wkv, bkv, Wo, bo, thr_s0, thr_o0):
    """Faithful host recompute of the collapsed recurrence (rows of the
    state are identical across n = (b, s) for any input, by induction
    from h0 = 0)."""
    x = np.asarray(x, np.float32)
    xt_all = np.moveaxis(x, 1, 0).reshape(T, N, D)
    decay = np.float32(MEM_DECAY)
    h = np.zeros(DS, np.float32)
    sv = np.zeros(DS, np.float32)
    ov = np.zeros(D, np.float32)
    ts = np.asarray(thr_s0, np.float32).copy()
    to = np.asarray(thr_o0, np.float32).copy()
    outs = np.zeros((T, D), np.float32)
    scale = np.float32(1.0 / math.sqrt(DH))
    for t in range(T):
        kv = xt_all[t] @ np.asarray(Wkv, np.float32).T + np.asarray(bkv, np.float32)
        k = kv[:, :DS].reshape(N, H, DH)
        v = kv[:, DS:].reshape(N, H, DH)
        q = (h @ np.asarray(Wq, np.float32).T + np.asarray(bq, np.float32)).reshape(H, DH)
        att = np.zeros((H, DH), np.float32)
        for hh in range(H):
            s = (k[:, hh, :] @ q[hh]) * scale
            w = _softmax_f32(s)
            att[hh] = w @ v[:, hh, :]
        su = h @ np.asarray(A, np.float32).T + att.reshape(DS) @ np.asarray(Wo, np.float32).T + np.asarray(bo, np.float32)
        vp = sv * decay + su
        spk = (vp - ts >= 0).astype(np.float32)
        sv = vp * (1 - spk)
        ts = np.maximum(ts + np.float32(ADAPT_STRENGTH) * (spk.mean(dtype=np.float32) - np.float32(TARGET_RATE)), np.float32(THR_MIN))
        h = spk
        op = h @ np.asarray(C, np.float32).T
        vpo = ov * decay + op
        spko = (vpo - to >= 0).astype(np.float32)
        ov = vpo * (1 - spko)
        to = np.maximum(to + np.float32(ADAPT_STRENGTH) * (spko.mean(dtype=np.float32) - np.float32(TARGET_RATE)), np.float32(THR_MIN))
        outs[t] = spko
    # broadcast the (identical) rows to the full output
    full = np.broadcast_to(outs[None, :, None, :], (B, T, S, D))
    return np.ascontiguousarray(full, dtype=np.float32)


def _margins_ok(partials_sum, Wo, bo, thr_s0):
    """Host verification of the no-spike hypothesis from the reduced
    attention partials.  Conservative: any margin within EPS_MARGIN of
    firing (or non-finite) rejects."""
    ps = partials_sum.astype(np.float64)
    asum = ps[0:DS, :]                      # (DS, T)
    wsum = ps[DS:DS + H, :]                 # (H, T)
    if not np.isfinite(ps).all() or (np.abs(wsum) < 1e-300).any():
        return False
    att_n = asum / np.repeat(wsum, DH, axis=0)
    su = np.asarray(Wo, np.float64) @ att_n + np.asarray(bo, np.float64)[:, None]
    if not np.isfinite(su).all():
        return False
    vp = np.zeros_like(su)
    acc = np.zeros(DS)
    for t in range(T):
        acc = acc * MEM_DECAY + su[:, t]
        vp[:, t] = acc
    ts0 = np.asarray(thr_s0, np.float64)
    thr = np.maximum(ts0[:, None] - ADAPT_STRENGTH * TARGET_RATE * np.arange(T)[None, :], THR_MIN)
    thr[:, 0] = ts0
    margin = vp - thr
    return np.isfinite(margin).all() and margin.max() < -EPS_MARGIN


def kernel(x, A, C, Wq, bq, Wkv, bkv, Wo, bo, thr_s0, thr_o0):
    x = np.ascontiguousarray(np.asarray(x, np.float32))
    bq = np.asarray(bq, np.float32)
    Wkv_ = np.asarray(Wkv, np.float32)
    bkv_ = np.asarray(bkv, np.float32)
    thr_s0 = np.asarray(thr_s0, np.float32)
    thr_o0 = np.asarray(thr_o0, np.float32)

    nc = _build_module()

    # host-side input marshaling (layout only)
    scale = np.float32(1.0 / math.sqrt(DH))
    qblk = np.zeros((DS, H), np.float32)
    for j in range(DS):
        qblk[j, j // DH] = bq[j] * scale
    patt = np.zeros((H, DS), np.float32)
    for j in range(DS):
        patt[j // DH, j] = 1.0
    consts = {
        "wkvt": np.ascontiguousarray(Wkv_.T),
        "bkv": bkv_.reshape(2 * DS, 1),
        "qblk": qblk,
        "patt": patt,
    }
    in_maps = []
    for c in range(NCORES):
        m = dict(consts)
        m["xt"] = np.ascontiguousarray(x[c].transpose(0, 2, 1))
        in_maps.append(m)

    res = bass_utils.run_bass_kernel_spmd(nc, in_maps, core_ids=list(range(NCORES)))

    partials_sum = np.sum(
        np.stack([r["partials"] for r in res.results]).astype(np.float64), axis=0)
    ok = (
        _margins_ok(partials_sum, Wo, bo, thr_s0)
        and float(thr_o0.min()) > EPS_MARGIN
    )
    if not ok:
        return _fallback(x, A, C, Wq, bq, Wkv, bkv, Wo, bo, thr_s0, thr_o0)

    # spike-free trajectory proved: output is the device-written zeros
    out = np.stack([r["out"] for r in res.results])  # (B, T, S, D)
    return np.ascontiguousarray(out, dtype=np.float32)


# revision 9
# speedup vs baseline: 2.2891x; 1.1474x over previous
"""Trainium2 Bass kernel for nn_AttentionalSpikingSSMLayer.

Model (reference semantics): a T-step scan; per step t:
    state_transition = h @ A.T
    q = h @ Wq.T + bq                  (queries from the spiking state)
    kv = x_t @ Wkv.T + bkv             (keys/values from the input)
    att = softmax(q k^T / sqrt(dh)) v  (attention over N = B*S, per head)
    state_update = state_transition + att @ Wo.T + bo
    h, v_mem_s, thr_s = LIF(state_update)          # binary spikes
    out_t, v_mem_o, thr_o = LIF(h @ C.T)           # binary spikes

Key structural algebra (exact, holds for ANY input values of these shapes):
  h0 = 0, and x enters only through k/v which are *reduced over* by
  attention.  Therefore every row n = (b,s) of the state performs the
  identical computation: h(t), v_mem(t) and the outputs are constant
  across (b, s) for all t.  The recurrence collapses to a single
  64/512-dim trajectory.

  Further, while no state spike fires, h(t) == 0, so q(t) == bq for all
  t.  The kernel exploits this speculatively: on device it computes the
  per-step attention sums for q = bq for all T steps in parallel
  (sharded over the N dimension across the 8 cores) and writes each
  core's partial sums; the host adds the 8 partials and verifies the
  no-spike hypothesis via the membrane-potential margins (v_pot - thr).
  If every margin is safely negative the hypothesis h == 0 is *proved*
  (by induction over t), the output is exactly zero, and the
  device-written zero tensor is the exact answer.  If any margin is
  within eps of firing (or non-finite), the host falls back to a
  faithful sequential recompute of the collapsed recurrence.

  The kv / scores / head-broadcast matmuls run in float32r (single-pass
  fp32, ~1e-4 relative rounding): the verification margin is O(1), so
  this cannot change any spike decision that the eps-guard would not
  already route to the exact fallback.
"""

import math
import numpy as np

import concourse.bass as bass
import concourse.tile as tile
from concourse import bacc, mybir
from concourse import bass_utils

F32 = mybir.dt.float32
F32R = mybir.dt.float32r

B, T, S, D = 8, 16, 256, 512
DS, H = 64, 4
DH = DS // H
N = B * S
NCORES = 8
MSH = N // NCORES          # 256 keys per core
TAU = 2.0
MEM_DECAY = math.exp(-1.0 / TAU)
ADAPT_STRENGTH = 0.1
TARGET_RATE = 0.02
THR_MIN = 0.5
EPS_MARGIN = 0.05          # conservative spike-detection margin
                           # (covers bf16 rounding of the speculative pass;
                           #  anything closer is recomputed exactly on host)

_CACHE = {}

TB = 4                     # timesteps batched per matmul (bf16 moving <= 1024)


def _build_module():
    """Build + compile the 8-core Bass module once per process."""
    if "nc" in _CACHE:
        return _CACHE["nc"]

    import ml_dtypes  # noqa: F401  (bf16 numpy dtype)
    BF16 = mybir.dt.bfloat16

    nc = bacc.Bacc("TRN2", target_bir_lowering=False, debug=False,
                   num_devices=NCORES)

    # x shard, pre-transposed to (T, D, m) and bf16 on host
    xt = nc.dram_tensor("xt", [T, D, MSH], BF16, kind="ExternalInput").ap()
    # fused [v-projection | per-head score-projection] weights (D, DS+H)
    wf = nc.dram_tensor("wf", [D, DS + H], BF16, kind="ExternalInput").ap()
    bf = nc.dram_tensor("bf", [DS + H, 1], F32, kind="ExternalInput").ap()
    patt = nc.dram_tensor("patt", [H, DS], BF16, kind="ExternalInput").ap()

    out = nc.dram_tensor("out", [T, S, D], F32, kind="ExternalOutput").ap()
    partials = nc.dram_tensor("partials", [DS + H, T], F32,
                              kind="ExternalOutput").ap()

    NB = T // TB
    with tile.TileContext(nc) as tc:
        with tc.tile_pool(name="const", bufs=1) as cpool, \
             tc.tile_pool(name="work", bufs=3) as wpool, \
             tc.tile_pool(name="ps", bufs=2, space="PSUM") as ps:

            # ---- constants (gpsimd ring; xt loads go on the sync ring) ----
            t_wf = cpool.tile([128, 4, DS + H], BF16)
            nc.gpsimd.dma_start(t_wf[:], wf.rearrange("(a p) m -> p a m", p=128))
            t_bf = cpool.tile([DS + H, 1], F32)
            nc.gpsimd.dma_start(t_bf[:], bf[:])
            t_patt = cpool.tile([H, DS], BF16)
            nc.gpsimd.dma_start(t_patt[:], patt[:])

            zt = cpool.tile([128, 4, D], F32)
            nc.vector.memset(zt[:], 0.0)

            attsums = cpool.tile([DS, T], F32)   # sum_m w * v   (per feature)
            wsums = cpool.tile([H, T], F32)      # sum_m w       (per head)
            for b in range(NB):
                t0 = b * TB
                # zero 2 output slabs per DMA on the gpsimd ring (overlaps
                # the sync-ring input loads)
                for z in range(TB // 2):
                    nc.gpsimd.dma_start(
                        out[t0 + 2 * z:t0 + 2 * z + 2].rearrange(
                            "t (a p) d -> p (t a) d", p=128),
                        zt[:])

                xt_b = wpool.tile([128, 4, TB, MSH], BF16, tag="xt")
                for a in range(4):
                    nc.sync.dma_start(
                        xt_b[:, a], xt[t0:t0 + TB, 128 * a:128 * (a + 1), :]
                        .rearrange("t p m -> p t m"))
                # fused [v | scores] = Wf^T @ x  for TB steps at once
                kv_ps = ps.tile([DS + H, TB * MSH], F32, tag="kv")
                xt_f = xt_b[:].rearrange("p a t m -> p a (t m)")
                for hh in range(2):
                    sl = slice(hh * TB * MSH // 2, (hh + 1) * TB * MSH // 2)
                    for a in range(4):
                        nc.tensor.matmul(kv_ps[:, sl], t_wf[:, a, :],
                                         xt_f[:, a, sl],
                                         start=(a == 0), stop=(a == 3))
                kvT = wpool.tile([DS + H, TB * MSH], BF16, tag="kvT")
                nc.vector.tensor_scalar(kvT[:], kv_ps[:], t_bf[:], None,
                                        op0=mybir.AluOpType.add)
                w = wpool.tile([H, TB * MSH], BF16, tag="w")
                nc.scalar.activation(w[:], kvT[DS:DS + H, :],
                                     mybir.ActivationFunctionType.Exp)
                # replicate head weights across the 16 features of each head
                wrep_ps = ps.tile([DS, TB * MSH], F32, tag="wrep")
                for hh in range(2):
                    sl = slice(hh * TB * MSH // 2, (hh + 1) * TB * MSH // 2)
                    nc.tensor.matmul(wrep_ps[:, sl], t_patt[:], w[:, sl],
                                     start=True, stop=True)
                scr = wpool.tile([DS, TB * MSH], BF16, tag="scr")
                nc.vector.tensor_mul(scr[:], kvT[0:DS, :], wrep_ps[:])
                nc.vector.reduce_sum(
                    out=attsums[:, t0:t0 + TB],
                    in_=scr[:].rearrange("p (t m) -> p t m", t=TB),
                    axis=mybir.AxisListType.X)
                nc.vector.reduce_sum(
                    out=wsums[:, t0:t0 + TB],
                    in_=w[:].rearrange("p (t m) -> p t m", t=TB),
                    axis=mybir.AxisListType.X)

            nc.sync.dma_start(partials[0:DS, :], attsums[:])
            nc.sync.dma_start(partials[DS:DS + H, :], wsums[:])

    nc.compile()
    _CACHE["nc"] = nc
    return nc


def _softmax_f32(s):
    m = s.max()
    e = np.exp(s - m, dtype=np.float32)
    return e / e.sum(dtype=np.float32)


def _fallback(x, A, C, Wq, bq, Wkv, bkv, Wo, bo, thr_s0, thr_o0):
    """Faithful host recompute of the collapsed recurrence (rows of the
    state are identical across n = (b, s) for any input, by induction
    from h0 = 0)."""
    x = np.asarray(x, np.float32)
    xt_all = np.moveaxis(x, 1, 0).reshape(T, N, D)
    decay = np.float32(MEM_DECAY)
    h = np.zeros(DS, np.float32)
    sv = np.zeros(DS, np.float32)
    ov = np.zeros(D, np.float32)
    ts = np.asarray(thr_s0, np.float32).copy()
    to = np.asarray(thr_o0, np.float32).copy()
    outs = np.zeros((T, D), np.float32)
    scale = np.float32(1.0 / math.sqrt(DH))
    for t in range(T):
        kv = xt_all[t] @ np.asarray(Wkv, np.float32).T + np.asarray(bkv, np.float32)
        k = kv[:, :DS].reshape(N, H, DH)
        v = kv[:, DS:].reshape(N, H, DH)
        q = (h @ np.asarray(Wq, np.float32).T + np.asarray(bq, np.float32)).reshape(H, DH)
        att = np.zeros((H, DH), np.float32)
        for hh in range(H):
            s = (k[:, hh, :] @ q[hh]) * scale
            w = _softmax_f32(s)
            att[hh] = w @ v[:, hh, :]
        su = h @ np.asarray(A, np.float32).T + att.reshape(DS) @ np.asarray(Wo, np.float32).T + np.asarray(bo, np.float32)
        vp = sv * decay + su
        spk = (vp - ts >= 0).astype(np.float32)
        sv = vp * (1 - spk)
        ts = np.maximum(ts + np.float32(ADAPT_STRENGTH) * (spk.mean(dtype=np.float32) - np.float32(TARGET_RATE)), np.float32(THR_MIN))
        h = spk
        op = h @ np.asarray(C, np.float32).T
        vpo = ov * decay + op
        spko = (vpo - to >= 0).astype(np.float32)
        ov = vpo * (1 - spko)
        to = np.maximum(to + np.float32(ADAPT_STRENGTH) * (spko.mean(dtype=np.float32) - np.float32(TARGET_RATE)), np.float32(THR_MIN))
        outs[t] = spko
    # broadcast the (identical) rows to the full output
    full = np.broadcast_to(outs[None, :, None, :], (B, T, S, D))
    return np.ascontiguousarray(full, dtype=np.float32)


def _margins_ok(partials_sum, Wo, bo, thr_s0):
    """Host verification of the no-spike hypothesis from the reduced
    attention partials.  Conservative: any margin within EPS_MARGIN of
    firing (or non-finite) rejects."""
    ps = partials_sum.astype(np.float64)
    asum = ps[0:DS, :]                      # (DS, T)
    wsum = ps[DS:DS + H, :]                 # (H, T)
    if not np.isfinite(ps).all() or (np.abs(wsum) < 1e-300).any():
        return False
    att_n = asum / np.repeat(wsum, DH, axis=0)
    su = np.asarray(Wo, np.float64) @ att_n + np.asarray(bo, np.float64)[:, None]
    if not np.isfinite(su).all():
        return False
    vp = np.zeros_like(su)
    acc = np.zeros(DS)
    for t in range(T):
        acc = acc * MEM_DECAY + su[:, t]
        vp[:, t] = acc
    ts0 = np.asarray(thr_s0, np.float64)
    thr = np.maximum(ts0[:, None] - ADAPT_STRENGTH * TARGET_RATE * np.arange(T)[None, :], THR_MIN)
    thr[:, 0] = ts0
    margin = vp - thr
    return np.isfinite(margin).all() and margin.max() < -EPS_MARGIN


def kernel(x, A, C, Wq, bq, Wkv, bkv, Wo, bo, thr_s0, thr_o0):
    x = np.ascontiguousarray(np.asarray(x, np.float32))
    bq = np.asarray(bq, np.float32)
    Wkv_ = np.asarray(Wkv, np.float32)
    bkv_ = np.asarray(bkv, np.float32)
    thr_s0 = np.asarray(thr_s0, np.float32)
    thr_o0 = np.asarray(thr_o0, np.float32)

    nc = _build_module()
    import ml_dtypes
    bf16 = ml_dtypes.bfloat16

    # host-side marshaling: layout + weight folding (scores = (bq^T Wk) x)
    scale = np.float32(1.0 / math.sqrt(DH))
    qblk = np.zeros((DS, H), np.float32)
    for j in range(DS):
        qblk[j, j // DH] = bq[j] * scale
    patt = np.zeros((H, DS), np.float32)
    for j in range(DS):
        patt[j // DH, j] = 1.0
    Wk = Wkv_[0:DS, :].astype(np.float64)      # (DS, D) key projection
    Wv = Wkv_[DS:2 * DS, :]                    # (DS, D) value projection
    Wsc = qblk.astype(np.float64).T @ Wk       # (H, D) folded score projection
    wf = np.concatenate([Wv.T, Wsc.T.astype(np.float32)], axis=1)  # (D, DS+H)
    bsc = qblk.astype(np.float64).T @ bkv_[0:DS].astype(np.float64)
    bfv = np.concatenate([bkv_[DS:2 * DS], bsc.astype(np.float32)]).reshape(DS + H, 1)
    consts = {
        "wf": wf.astype(bf16),
        "bf": bfv.astype(np.float32),
        "patt": patt.astype(bf16),
    }
    in_maps = []
    for c in range(NCORES):
        m = dict(consts)
        m["xt"] = np.ascontiguousarray(x[c].transpose(0, 2, 1)).astype(bf16)
        in_maps.append(m)

    res = bass_utils.run_bass_kernel_spmd(nc, in_maps, core_ids=list(range(NCORES)))

    partials_sum = np.sum(
        np.stack([r["partials"] for r in res.results]).astype(np.float64), axis=0)
    ok = (
        _margins_ok(partials_sum, Wo, bo, thr_s0)
        and float(thr_o0.min()) > EPS_MARGIN
    )
    if not ok:
        return _fallback(x, A, C, Wq, bq, Wkv, bkv, Wo, bo, thr_s0, thr_o0)

    # spike-free trajectory proved: output is the device-written zeros
    out = np.stack([r["out"] for r in res.results])  # (B, T, S, D)
    return np.ascontiguousarray(out, dtype=np.float32)


# revision 11
# speedup vs baseline: 2.7424x; 1.1980x over previous
"""Trainium2 Bass kernel for nn_AttentionalSpikingSSMLayer.

Model (reference semantics): a T-step scan; per step t:
    state_transition = h @ A.T
    q = h @ Wq.T + bq                  (queries from the spiking state)
    kv = x_t @ Wkv.T + bkv             (keys/values from the input)
    att = softmax(q k^T / sqrt(dh)) v  (attention over N = B*S, per head)
    state_update = state_transition + att @ Wo.T + bo
    h, v_mem_s, thr_s = LIF(state_update)          # binary spikes
    out_t, v_mem_o, thr_o = LIF(h @ C.T)           # binary spikes

Key structural algebra (exact, holds for ANY input values of these shapes):
  h0 = 0, and x enters only through k/v which are *reduced over* by
  attention.  Therefore every row n = (b,s) of the state performs the
  identical computation: h(t), v_mem(t) and the outputs are constant
  across (b, s) for all t.  The recurrence collapses to a single
  64/512-dim trajectory.

  Further, while no state spike fires, h(t) == 0, so q(t) == bq for all
  t.  The kernel exploits this speculatively: on device it computes the
  per-step attention sums for q = bq for all T steps in parallel
  (sharded over the N dimension across the 8 cores) and writes each
  core's partial sums; the host adds the 8 partials and verifies the
  no-spike hypothesis via the membrane-potential margins (v_pot - thr).
  If every margin is safely negative the hypothesis h == 0 is *proved*
  (by induction over t), the output is exactly zero, and the
  device-written zero tensor is the exact answer.  If any margin is
  within eps of firing (or non-finite), the host falls back to a
  faithful sequential recompute of the collapsed recurrence.

  The kv / scores / head-broadcast matmuls run in float32r (single-pass
  fp32, ~1e-4 relative rounding): the verification margin is O(1), so
  this cannot change any spike decision that the eps-guard would not
  already route to the exact fallback.
"""

import math
import numpy as np

import concourse.bass as bass
import concourse.tile as tile
from concourse import bacc, mybir
from concourse import bass_utils

F32 = mybir.dt.float32
F32R = mybir.dt.float32r

B, T, S, D = 8, 16, 256, 512
DS, H = 64, 4
DH = DS // H
N = B * S
NCORES = 8
MSH = N // NCORES          # 256 keys per core
TAU = 2.0
MEM_DECAY = math.exp(-1.0 / TAU)
ADAPT_STRENGTH = 0.1
TARGET_RATE = 0.02
THR_MIN = 0.5
EPS_MARGIN = 0.05          # conservative spike-detection margin
                           # (covers bf16 rounding of the speculative pass;
                           #  anything closer is recomputed exactly on host)

_CACHE = {}

TB = 4                     # timesteps batched per matmul (bf16 moving <= 1024)


def _build_module():
    """Build + compile the 8-core Bass module once per process."""
    if "nc" in _CACHE:
        return _CACHE["nc"]

    import ml_dtypes  # noqa: F401  (bf16 numpy dtype)
    BF16 = mybir.dt.bfloat16
    NU = 8                 # pipeline units
    TU = T // NU           # timesteps per unit
    FR = TU * MSH          # free size per unit (512)

    nc = bacc.Bacc("TRN2", target_bir_lowering=False, debug=False,
                   num_devices=NCORES)

    # x shard: host pre-reshaped to (unit, partition, ktile, t, m), bf16
    xt = nc.dram_tensor("xt", [NU, 128, 4, TU, MSH], BF16,
                        kind="ExternalInput").ap()
    # fused weights (D, 100): [v (64) | ones (4) | pad (28) | score-proj (4)]
    # (scores sit at partition 96 so the exp slice starts on a quad boundary)
    wf = nc.dram_tensor("wf", [D, 100], BF16, kind="ExternalInput").ap()
    bf = nc.dram_tensor("bf", [100, 1], F32, kind="ExternalInput").ap()
    patt = nc.dram_tensor("patt", [H, DS + H], BF16, kind="ExternalInput").ap()

    out = nc.dram_tensor("out", [T, S, D], F32, kind="ExternalOutput").ap()
    partials = nc.dram_tensor("partials", [DS + H, T], F32,
                              kind="ExternalOutput").ap()

    with tile.TileContext(nc) as tc:
        with tc.tile_pool(name="const", bufs=1) as cpool, \
             tc.tile_pool(name="work", bufs=3) as wpool, \
             tc.tile_pool(name="psA", bufs=3, space="PSUM") as psA, \
             tc.tile_pool(name="psB", bufs=3, space="PSUM") as psB:

            # ---- constants (gpsimd ring, ahead of the xt stream) ----
            t_wf = cpool.tile([128, 4, 100], BF16)
            nc.gpsimd.dma_start(t_wf[:], wf.rearrange("(a p) m -> p a m", p=128))
            t_bf = cpool.tile([100, 1], F32)
            nc.gpsimd.dma_start(t_bf[:], bf[:])
            t_patt = cpool.tile([H, DS + H], BF16)
            nc.gpsimd.dma_start(t_patt[:], patt[:])

            zt = cpool.tile([128, 8, D], F32)
            nc.vector.memset(zt[:], 0.0)
            # zero the output (4 x 2MB on the sync ring; overlaps compute)
            for z in range(4):
                nc.sync.dma_start(
                    out[4 * z:4 * z + 4].rearrange("t (a p) d -> p (t a) d", p=128),
                    zt[:])

            # combined [sum_m w*v (64) | sum_m w (4)] per step
            attw = cpool.tile([DS + H, T], F32)
            for u in range(NU):
                xt_u = wpool.tile([128, 4, TU, MSH], BF16, tag="xt")
                nc.gpsimd.dma_start(xt_u[:], xt[u])
                # fused [v | ones | scores] = Wf^T @ x for TU steps
                kv_ps = psA.tile([100, FR], F32, tag="kv")
                xt_f = xt_u[:].rearrange("p a t m -> p a (t m)")
                for a in range(4):
                    nc.tensor.matmul(kv_ps[:], t_wf[:, a, :], xt_f[:, a, :],
                                     start=(a == 0), stop=(a == 3))
                kvT = wpool.tile([100, FR], BF16, tag="kvT")
                nc.vector.tensor_scalar(kvT[:], kv_ps[:], t_bf[:], None,
                                        op0=mybir.AluOpType.add)
                w = wpool.tile([H, FR], BF16, tag="w")
                nc.scalar.activation(w[:], kvT[96:100, :],
                                     mybir.ActivationFunctionType.Exp)
                # head weights replicated over [16 features per head | head one-col]
                wrep_ps = psB.tile([DS + H, FR], F32, tag="wrep")
                nc.tensor.matmul(wrep_ps[:], t_patt[:], w[:], start=True, stop=True)
                scr = wpool.tile([DS + H, FR], BF16, tag="scr")
                nc.vector.tensor_mul(scr[:], kvT[0:DS + H, :], wrep_ps[:])
                nc.vector.reduce_sum(
                    out=attw[:, u * TU:(u + 1) * TU],
                    in_=scr[:].rearrange("p (t m) -> p t m", t=TU),
                    axis=mybir.AxisListType.X)

            nc.sync.dma_start(partials[:], attw[:])

    nc.compile()
    _CACHE["nc"] = nc
    return nc


def _softmax_f32(s):
    m = s.max()
    e = np.exp(s - m, dtype=np.float32)
    return e / e.sum(dtype=np.float32)


def _fallback(x, A, C, Wq, bq, Wkv, bkv, Wo, bo, thr_s0, thr_o0):
    """Faithful host recompute of the collapsed recurrence (rows of the
    state are identical across n = (b, s) for any input, by induction
    from h0 = 0)."""
    x = np.asarray(x, np.float32)
    xt_all = np.moveaxis(x, 1, 0).reshape(T, N, D)
    decay = np.float32(MEM_DECAY)
    h = np.zeros(DS, np.float32)
    sv = np.zeros(DS, np.float32)
    ov = np.zeros(D, np.float32)
    ts = np.asarray(thr_s0, np.float32).copy()
    to = np.asarray(thr_o0, np.float32).copy()
    outs = np.zeros((T, D), np.float32)
    scale = np.float32(1.0 / math.sqrt(DH))
    for t in range(T):
        kv = xt_all[t] @ np.asarray(Wkv, np.float32).T + np.asarray(bkv, np.float32)
        k = kv[:, :DS].reshape(N, H, DH)
        v = kv[:, DS:].reshape(N, H, DH)
        q = (h @ np.asarray(Wq, np.float32).T + np.asarray(bq, np.float32)).reshape(H, DH)
        att = np.zeros((H, DH), np.float32)
        for hh in range(H):
            s = (k[:, hh, :] @ q[hh]) * scale
            w = _softmax_f32(s)
            att[hh] = w @ v[:, hh, :]
        su = h @ np.asarray(A, np.float32).T + att.reshape(DS) @ np.asarray(Wo, np.float32).T + np.asarray(bo, np.float32)
        vp = sv * decay + su
        spk = (vp - ts >= 0).astype(np.float32)
        sv = vp * (1 - spk)
        ts = np.maximum(ts + np.float32(ADAPT_STRENGTH) * (spk.mean(dtype=np.float32) - np.float32(TARGET_RATE)), np.float32(THR_MIN))
        h = spk
        op = h @ np.asarray(C, np.float32).T
        vpo = ov * decay + op
        spko = (vpo - to >= 0).astype(np.float32)
        ov = vpo * (1 - spko)
        to = np.maximum(to + np.float32(ADAPT_STRENGTH) * (spko.mean(dtype=np.float32) - np.float32(TARGET_RATE)), np.float32(THR_MIN))
        outs[t] = spko
    # broadcast the (identical) rows to the full output
    full = np.broadcast_to(outs[None, :, None, :], (B, T, S, D))
    return np.ascontiguousarray(full, dtype=np.float32)


def _margins_ok(partials_sum, Wo, bo, thr_s0):
    """Host verification of the no-spike hypothesis from the reduced
    attention partials.  Conservative: any margin within EPS_MARGIN of
    firing (or non-finite) rejects."""
    ps = partials_sum.astype(np.float64)
    asum = ps[0:DS, :]                      # (DS, T)
    wsum = ps[DS:DS + H, :]                 # (H, T)
    if not np.isfinite(ps).all() or (np.abs(wsum) < 1e-300).any():
        return False
    att_n = asum / np.repeat(wsum, DH, axis=0)
    su = np.asarray(Wo, np.float64) @ att_n + np.asarray(bo, np.float64)[:, None]
    if not np.isfinite(su).all():
        return False
    vp = np.zeros_like(su)
    acc = np.zeros(DS)
    for t in range(T):
        acc = acc * MEM_DECAY + su[:, t]
        vp[:, t] = acc
    ts0 = np.asarray(thr_s0, np.float64)
    thr = np.maximum(ts0[:, None] - ADAPT_STRENGTH * TARGET_RATE * np.arange(T)[None, :], THR_MIN)
    thr[:, 0] = ts0
    margin = vp - thr
    return np.isfinite(margin).all() and margin.max() < -EPS_MARGIN


def kernel(x, A, C, Wq, bq, Wkv, bkv, Wo, bo, thr_s0, thr_o0):
    x = np.ascontiguousarray(np.asarray(x, np.float32))
    bq = np.asarray(bq, np.float32)
    Wkv_ = np.asarray(Wkv, np.float32)
    bkv_ = np.asarray(bkv, np.float32)
    thr_s0 = np.asarray(thr_s0, np.float32)
    thr_o0 = np.asarray(thr_o0, np.float32)

    nc = _build_module()
    import ml_dtypes
    bf16 = ml_dtypes.bfloat16

    # host-side marshaling: layout + weight folding (scores = (bq^T Wk) x)
    scale = np.float32(1.0 / math.sqrt(DH))
    qblk = np.zeros((DS, H), np.float32)
    for j in range(DS):
        qblk[j, j // DH] = bq[j] * scale
    # patt covers [64 features -> head | 4 one-columns -> head]
    patt = np.zeros((H, DS + H), np.float32)
    for j in range(DS):
        patt[j // DH, j] = 1.0
    for h in range(H):
        patt[h, DS + h] = 1.0
    Wk = Wkv_[0:DS, :].astype(np.float64)      # (DS, D) key projection
    Wv = Wkv_[DS:2 * DS, :]                    # (DS, D) value projection
    Wsc = qblk.astype(np.float64).T @ Wk       # (H, D) folded score projection
    wf = np.concatenate(
        [Wv.T, np.zeros((D, 32), np.float32), Wsc.T.astype(np.float32)], axis=1)
    bsc = qblk.astype(np.float64).T @ bkv_[0:DS].astype(np.float64)
    bfv = np.concatenate(
        [bkv_[DS:2 * DS], np.ones(H, np.float32), np.zeros(28, np.float32),
         bsc.astype(np.float32)]).reshape(100, 1)
    consts = {
        "wf": wf.astype(bf16),
        "bf": bfv.astype(np.float32),
        "patt": patt.astype(bf16),
    }
    NU, TU = 8, T // 8
    in_maps = []
    for c in range(NCORES):
        m = dict(consts)
        xtc = x[c].transpose(0, 2, 1)                      # (T, D, MSH)
        xtc = xtc.reshape(NU, TU, 4, 128, MSH).transpose(0, 3, 2, 1, 4)
        m["xt"] = np.ascontiguousarray(xtc).astype(bf16)   # (NU,128,4,TU,MSH)
        in_maps.append(m)

    res = bass_utils.run_bass_kernel_spmd(nc, in_maps, core_ids=list(range(NCORES)))

    partials_sum = np.sum(
        np.stack([r["partials"] for r in res.results]).astype(np.float64), axis=0)
    ok = (
        _margins_ok(partials_sum, Wo, bo, thr_s0)
        and float(thr_o0.min()) > EPS_MARGIN
    )
    if not ok:
        return _fallback(x, A, C, Wq, bq, Wkv, bkv, Wo, bo, thr_s0, thr_o0)

    # spike-free trajectory proved: output is the device-written zeros
    out = np.stack([r["out"] for r in res.results])  # (B, T, S, D)
    return np.ascontiguousarray(out, dtype=np.float32)


# revision 12
# speedup vs baseline: 3.1798x; 1.1595x over previous
"""Trainium2 Bass kernel for nn_AttentionalSpikingSSMLayer.

Model (reference semantics): a T-step scan; per step t:
    state_transition = h @ A.T
    q = h @ Wq.T + bq                  (queries from the spiking state)
    kv = x_t @ Wkv.T + bkv             (keys/values from the input)
    att = softmax(q k^T / sqrt(dh)) v  (attention over N = B*S, per head)
    state_update = state_transition + att @ Wo.T + bo
    h, v_mem_s, thr_s = LIF(state_update)          # binary spikes
    out_t, v_mem_o, thr_o = LIF(h @ C.T)           # binary spikes

Key structural algebra (exact, holds for ANY input values of these shapes):
  h0 = 0, and x enters only through k/v which are *reduced over* by
  attention.  Therefore every row n = (b,s) of the state performs the
  identical computation: h(t), v_mem(t) and the outputs are constant
  across (b, s) for all t.  The recurrence collapses to a single
  64/512-dim trajectory.

  Further, while no state spike fires, h(t) == 0, so q(t) == bq for all
  t.  The kernel exploits this speculatively: on device it computes the
  per-step attention sums for q = bq for all T steps in parallel
  (sharded over the N dimension across the 8 cores) and writes each
  core's partial sums; the host adds the 8 partials and verifies the
  no-spike hypothesis via the membrane-potential margins (v_pot - thr).
  If every margin is safely negative the hypothesis h == 0 is *proved*
  (by induction over t), the output is exactly zero, and the
  device-written zero tensor is the exact answer.  If any margin is
  within eps of firing (or non-finite), the host falls back to a
  faithful sequential recompute of the collapsed recurrence.

  The kv / scores / head-broadcast matmuls run in float32r (single-pass
  fp32, ~1e-4 relative rounding): the verification margin is O(1), so
  this cannot change any spike decision that the eps-guard would not
  already route to the exact fallback.
"""

import math
import numpy as np

import concourse.bass as bass
import concourse.tile as tile
from concourse import bacc, mybir
from concourse import bass_utils

F32 = mybir.dt.float32
F32R = mybir.dt.float32r

B, T, S, D = 8, 16, 256, 512
DS, H = 64, 4
DH = DS // H
N = B * S
NCORES = 8
MSH = N // NCORES          # 256 keys per core
TAU = 2.0
MEM_DECAY = math.exp(-1.0 / TAU)
ADAPT_STRENGTH = 0.1
TARGET_RATE = 0.02
THR_MIN = 0.5
EPS_MARGIN = 0.05          # conservative spike-detection margin
                           # (covers bf16 rounding of the speculative pass;
                           #  anything closer is recomputed exactly on host)

_CACHE = {}

TB = 4                     # timesteps batched per matmul (bf16 moving <= 1024)


def _build_module():
    """Build + compile the 8-core Bass module once per process."""
    if "nc" in _CACHE:
        return _CACHE["nc"]

    import ml_dtypes  # noqa: F401  (bf16 numpy dtype)
    BF16 = mybir.dt.bfloat16
    NU = 8                 # pipeline units
    TU = T // NU           # timesteps per unit
    FR = TU * MSH          # free size per unit (512)

    nc = bacc.Bacc("TRN2", target_bir_lowering=False, debug=False,
                   num_devices=NCORES)

    # x shard: host pre-reshaped to (unit, partition, ktile, t, m), bf16
    xt = nc.dram_tensor("xt", [NU, 128, 4, TU, MSH], BF16,
                        kind="ExternalInput").ap()
    # fused weights (D, 100): [v (64) | ones (4) | pad (28) | score-proj (4)]
    # (scores sit at partition 96 so the exp slice starts on a quad boundary)
    wf = nc.dram_tensor("wf", [D, 128], BF16, kind="ExternalInput").ap()
    bf = nc.dram_tensor("bf", [128, 1], F32, kind="ExternalInput").ap()
    patt = nc.dram_tensor("patt", [H, 128], BF16, kind="ExternalInput").ap()

    out = nc.dram_tensor("out", [T, S, D], F32, kind="ExternalOutput").ap()
    partials = nc.dram_tensor("partials", [DS + H, T], F32,
                              kind="ExternalOutput").ap()

    with tile.TileContext(nc) as tc:
        with tc.tile_pool(name="const", bufs=1) as cpool, \
             tc.tile_pool(name="work", bufs=3) as wpool, \
             tc.tile_pool(name="psA", bufs=3, space="PSUM") as psA, \
             tc.tile_pool(name="psB", bufs=3, space="PSUM") as psB:

            # ---- constants (gpsimd ring, ahead of the xt stream) ----
            t_wf = cpool.tile([128, 4, 128], BF16)
            nc.gpsimd.dma_start(t_wf[:], wf.rearrange("(a p) m -> p a m", p=128))
            t_bf = cpool.tile([128, 1], F32)
            nc.gpsimd.dma_start(t_bf[:], bf[:])
            t_patt = cpool.tile([H, 128], BF16)
            nc.gpsimd.dma_start(t_patt[:], patt[:])

            zt = cpool.tile([128, 8, D], F32)
            nc.vector.memset(zt[:], 0.0)
            # zero the output (4 x 2MB on the sync ring; overlaps compute)
            for z in range(4):
                nc.sync.dma_start(
                    out[4 * z:4 * z + 4].rearrange("t (a p) d -> p (t a) d", p=128),
                    zt[:])

            # combined [sum_m w*v (64) | sum_m w (4)] per step
            attw = cpool.tile([DS + H, T], F32)
            for u in range(NU):
                xt_u = wpool.tile([128, 4, TU, MSH], BF16, tag="xt")
                nc.gpsimd.dma_start(xt_u[:], xt[u])
                # fused [v | ones | scores] = Wf^T @ x for TU steps
                kv_ps = psA.tile([128, FR], F32, tag="kv")
                xt_f = xt_u[:].rearrange("p a t m -> p a (t m)")
                for a in range(4):
                    nc.tensor.matmul(kv_ps[:], t_wf[:, a, :], xt_f[:, a, :],
                                     start=(a == 0), stop=(a == 3))
                kvT = wpool.tile([128, FR], BF16, tag="kvT")
                nc.vector.tensor_scalar(kvT[:], kv_ps[:], t_bf[:], None,
                                        op0=mybir.AluOpType.add)
                w = wpool.tile([H, FR], BF16, tag="w")
                nc.scalar.activation(w[:], kvT[96:100, :],
                                     mybir.ActivationFunctionType.Exp)
                # head weights replicated over [16 features per head | head one-col]
                wrep_ps = psB.tile([128, FR], F32, tag="wrep")
                nc.tensor.matmul(wrep_ps[:], t_patt[:], w[:], start=True, stop=True)
                scr = wpool.tile([DS + H, FR], BF16, tag="scr")
                nc.vector.tensor_mul(scr[:], kvT[0:DS + H, :], wrep_ps[0:DS + H, :])
                nc.vector.reduce_sum(
                    out=attw[:, u * TU:(u + 1) * TU],
                    in_=scr[:].rearrange("p (t m) -> p t m", t=TU),
                    axis=mybir.AxisListType.X)

            nc.sync.dma_start(partials[:], attw[:])

    nc.compile()
    _CACHE["nc"] = nc
    return nc


def _softmax_f32(s):
    m = s.max()
    e = np.exp(s - m, dtype=np.float32)
    return e / e.sum(dtype=np.float32)


def _fallback(x, A, C, Wq, bq, Wkv, bkv, Wo, bo, thr_s0, thr_o0):
    """Faithful host recompute of the collapsed recurrence (rows of the
    state are identical across n = (b, s) for any input, by induction
    from h0 = 0)."""
    x = np.asarray(x, np.float32)
    xt_all = np.moveaxis(x, 1, 0).reshape(T, N, D)
    decay = np.float32(MEM_DECAY)
    h = np.zeros(DS, np.float32)
    sv = np.zeros(DS, np.float32)
    ov = np.zeros(D, np.float32)
    ts = np.asarray(thr_s0, np.float32).copy()
    to = np.asarray(thr_o0, np.float32).copy()
    outs = np.zeros((T, D), np.float32)
    scale = np.float32(1.0 / math.sqrt(DH))
    for t in range(T):
        kv = xt_all[t] @ np.asarray(Wkv, np.float32).T + np.asarray(bkv, np.float32)
        k = kv[:, :DS].reshape(N, H, DH)
        v = kv[:, DS:].reshape(N, H, DH)
        q = (h @ np.asarray(Wq, np.float32).T + np.asarray(bq, np.float32)).reshape(H, DH)
        att = np.zeros((H, DH), np.float32)
        for hh in range(H):
            s = (k[:, hh, :] @ q[hh]) * scale
            w = _softmax_f32(s)
            att[hh] = w @ v[:, hh, :]
        su = h @ np.asarray(A, np.float32).T + att.reshape(DS) @ np.asarray(Wo, np.float32).T + np.asarray(bo, np.float32)
        vp = sv * decay + su
        spk = (vp - ts >= 0).astype(np.float32)
        sv = vp * (1 - spk)
        ts = np.maximum(ts + np.float32(ADAPT_STRENGTH) * (spk.mean(dtype=np.float32) - np.float32(TARGET_RATE)), np.float32(THR_MIN))
        h = spk
        op = h @ np.asarray(C, np.float32).T
        vpo = ov * decay + op
        spko = (vpo - to >= 0).astype(np.float32)
        ov = vpo * (1 - spko)
        to = np.maximum(to + np.float32(ADAPT_STRENGTH) * (spko.mean(dtype=np.float32) - np.float32(TARGET_RATE)), np.float32(THR_MIN))
        outs[t] = spko
    # broadcast the (identical) rows to the full output
    full = np.broadcast_to(outs[None, :, None, :], (B, T, S, D))
    return np.ascontiguousarray(full, dtype=np.float32)


def _margins_ok(partials_sum, Wo, bo, thr_s0):
    """Host verification of the no-spike hypothesis from the reduced
    attention partials.  Conservative: any margin within EPS_MARGIN of
    firing (or non-finite) rejects."""
    ps = partials_sum.astype(np.float64)
    asum = ps[0:DS, :]                      # (DS, T)
    wsum = ps[DS:DS + H, :]                 # (H, T)
    if not np.isfinite(ps).all() or (np.abs(wsum) < 1e-300).any():
        return False
    att_n = asum / np.repeat(wsum, DH, axis=0)
    su = np.asarray(Wo, np.float64) @ att_n + np.asarray(bo, np.float64)[:, None]
    if not np.isfinite(su).all():
        return False
    vp = np.zeros_like(su)
    acc = np.zeros(DS)
    for t in range(T):
        acc = acc * MEM_DECAY + su[:, t]
        vp[:, t] = acc
    ts0 = np.asarray(thr_s0, np.float64)
    thr = np.maximum(ts0[:, None] - ADAPT_STRENGTH * TARGET_RATE * np.arange(T)[None, :], THR_MIN)
    thr[:, 0] = ts0
    margin = vp - thr
    return np.isfinite(margin).all() and margin.max() < -EPS_MARGIN


def kernel(x, A, C, Wq, bq, Wkv, bkv, Wo, bo, thr_s0, thr_o0):
    x = np.ascontiguousarray(np.asarray(x, np.float32))
    bq = np.asarray(bq, np.float32)
    Wkv_ = np.asarray(Wkv, np.float32)
    bkv_ = np.asarray(bkv, np.float32)
    thr_s0 = np.asarray(thr_s0, np.float32)
    thr_o0 = np.asarray(thr_o0, np.float32)

    nc = _build_module()
    import ml_dtypes
    bf16 = ml_dtypes.bfloat16

    # host-side marshaling: layout + weight folding (scores = (bq^T Wk) x)
    scale = np.float32(1.0 / math.sqrt(DH))
    qblk = np.zeros((DS, H), np.float32)
    for j in range(DS):
        qblk[j, j // DH] = bq[j] * scale
    # patt covers [64 features -> head | 4 one-columns -> head]
    patt = np.zeros((H, 128), np.float32)
    for j in range(DS):
        patt[j // DH, j] = 1.0
    for h in range(H):
        patt[h, DS + h] = 1.0
    Wk = Wkv_[0:DS, :].astype(np.float64)      # (DS, D) key projection
    Wv = Wkv_[DS:2 * DS, :]                    # (DS, D) value projection
    Wsc = qblk.astype(np.float64).T @ Wk       # (H, D) folded score projection
    wf = np.concatenate(
        [Wv.T, np.zeros((D, 32), np.float32), Wsc.T.astype(np.float32),
         np.zeros((D, 28), np.float32)], axis=1)
    bsc = qblk.astype(np.float64).T @ bkv_[0:DS].astype(np.float64)
    bfv = np.concatenate(
        [bkv_[DS:2 * DS], np.ones(H, np.float32), np.zeros(28, np.float32),
         bsc.astype(np.float32), np.zeros(28, np.float32)]).reshape(128, 1)
    consts = {
        "wf": wf.astype(bf16),
        "bf": bfv.astype(np.float32),
        "patt": patt.astype(bf16),
    }
    NU, TU = 8, T // 8
    in_maps = []
    for c in range(NCORES):
        m = dict(consts)
        xtc = x[c].transpose(0, 2, 1)                      # (T, D, MSH)
        xtc = xtc.reshape(NU, TU, 4, 128, MSH).transpose(0, 3, 2, 1, 4)
        m["xt"] = np.ascontiguousarray(xtc).astype(bf16)   # (NU,128,4,TU,MSH)
        in_maps.append(m)

    res = bass_utils.run_bass_kernel_spmd(nc, in_maps, core_ids=list(range(NCORES)))

    partials_sum = np.sum(
        np.stack([r["partials"] for r in res.results]).astype(np.float64), axis=0)
    ok = (
        _margins_ok(partials_sum, Wo, bo, thr_s0)
        and float(thr_o0.min()) > EPS_MARGIN
    )
    if not ok:
        return _fallback(x, A, C, Wq, bq, Wkv, bkv, Wo, bo, thr_s0, thr_o0)

    # spike-free trajectory proved: output is the device-written zeros
    out = np.stack([r["out"] for r in res.results])  # (B, T, S, D)
    return np.ascontiguousarray(out, dtype=np.float32)
